# revision 2
# baseline (speedup 1.0000x reference)
"""Multi-head causal attention (B=4, T=2048, C=1024, H=16) on 8 TRN2 NeuronCores.

Sharding: core c handles batch b = c//2 and head-group hg = c%2 (8 heads each),
Megatron-style. Each core computes its QKV projection slice, attention for its
8 heads, and a partial fc_out over its 512 input channels. The fc_out
all-reduce (2 cores per batch) and the +b_out happen on host.

v2 speedups over the bf16/f32r baseline:
- QKV projection chains run in fp8e4: two non-DR matmuls (ci 0,1; the first
  is the bank's single start=True covering all 512 columns — hardware
  start_tensor_calc pending-zeroes the whole PSUM bank, so any later
  start=True in the bank corrupts interleaved accumulation) followed by three
  DoubleRow pairs (ci 2..7, 2x128 contraction each, ~116 ns per 256-col
  matmul with an explicit ldweights). ~1186 ns per chain vs 1960 bf16.
- AV for q-chunks 1-3 runs in fp8: p is written as fp8 directly by the exp
  activation; V is stored fp8 as [128 kpos, head, unit, 128pad] so a DR
  matmul contracts 256 k-positions; the ones column accumulates the softmax
  denominator. The first unit-group per (pair, half) uses two non-DR fp8
  matmuls (single start=True over [0:512)); later groups accumulate via DR.
- q-chunk 0 (tokens 0-511) keeps a clean bf16 AV path (separate bf16 V store
  fed by bf16 V-chains from bf16 x/Wv): its rows average only a few V rows,
  so fp8 V/p noise passes straight through to the output absmax there.
- A and fc_out stay bf16 (fp8 A or W_out alone costs ~2-3e-2 absmax rel).
- Scores stay bf16 (DH=64 contraction is half-rate in any dtype); scores
  PSUM is double-buffered so scores(u+1) overlaps exp(u); output DMA is bf16.
"""

import hashlib
import numpy as np
from contextlib import ExitStack

import ml_dtypes
import concourse.tile as tile
from concourse import bacc, mybir
from concourse.alu_op_type import AluOpType as AluOp
from concourse.bass_utils import run_bass_kernel_spmd

B, T, C = 4, 2048, 1024
H, DH = 16, 64
NCORES = 8
QW = 512     # q-chunk width
KW = 128     # k-tile height
NQC = T // QW      # 4 q-chunks
NKT = T // KW      # 16 k-tiles
HPC = H // 2       # 8 heads per core
MPC = HPC * DH     # 512 qkv dims per core per projection
NC_T = C // 128    # 8 contraction tiles for x/W
NM_T = MPC // 128  # 4 m-tiles per projection
WSCALE = 16.0      # fp8 weight pre-scale (keeps U(+-1/32) out of subnormals)

f32 = mybir.dt.float32
bf16 = mybir.dt.bfloat16
fp8 = mybir.dt.float8e4
DRM = mybir.MatmulPerfMode.DoubleRow
Exp = mybir.ActivationFunctionType.Exp
E4 = ml_dtypes.float8_e4m3

_prog_cache: dict = {}
import os as _os
DEBUG = _os.environ.get("KDEBUG", "") == "1"



def _mask_plan(mask2d: np.ndarray):
    """Per q-chunk list of (kt, qoff, mask_idx, mc0, mc1) units + unique tiles.

    qoff: first q column (relative to chunk) with any unmasked k in the unit.
    [mc0, mc1): column range needing an explicit mask multiply after exp.
    """
    m = mask2d != 0
    units_by_qc = []
    mask_tiles: list[np.ndarray] = []
    tile_index: dict[bytes, int] = {}
    for qc in range(NQC):
        units = []
        for kt in range(NKT):
            blk = m[qc * QW:(qc + 1) * QW, kt * KW:(kt + 1) * KW]
            colany = blk.any(axis=1)
            if not colany.any():
                continue
            qoff = int(np.argmax(colany))
            colall = blk.all(axis=1)
            nontriv = np.nonzero(~colall[qoff:])[0]
            if len(nontriv):
                mc0 = qoff + int(nontriv[0])
                mc1 = qoff + int(nontriv[-1]) + 1
                mt = np.ascontiguousarray(blk.T).astype(np.float32)
                key = mt.tobytes()
                if key not in tile_index:
                    tile_index[key] = len(mask_tiles)
                    mask_tiles.append(mt)
                midx = tile_index[key]
            else:
                midx, mc0, mc1 = None, 0, 0
            units.append((kt, qoff, midx, mc0, mc1))
        units.sort(key=lambda u: (u[1], u[0]))
        units_by_qc.append(units)
    return units_by_qc, mask_tiles


def _pair_groups(units):
    """Pair consecutive units into DR groups; group = (ua, ub|None)."""
    groups = []
    i = 0
    while i < len(units):
        ua = units[i]
        ub = units[i + 1] if i + 1 < len(units) else None
        if ub is not None and ua[0] % 2 == 0 and ub[0] == ua[0] + 1:
            groups.append((ua, ub))
            i += 2
        else:
            groups.append((ua, None))
            i += 1
    return groups


def _build_program(units_by_qc, n_masks: int):
    nc = bacc.Bacc("TRN2", target_bir_lowering=False, debug=False,
                   enable_asserts=False)
    xt_d = nc.dram_tensor("xt", [C, T], fp8, kind="ExternalInput").ap()
    x0b_d = nc.dram_tensor("x0b", [C, QW], bf16, kind="ExternalInput").ap()
    wq_d = nc.dram_tensor("wq", [C, MPC], fp8, kind="ExternalInput").ap()
    wk_d = nc.dram_tensor("wk", [C, MPC], fp8, kind="ExternalInput").ap()
    wv_d = nc.dram_tensor("wv", [C, MPC], fp8, kind="ExternalInput").ap()
    wvb_d = nc.dram_tensor("wvb", [C, MPC], bf16, kind="ExternalInput").ap()
    bqk_d = nc.dram_tensor("bqk", [128, 8], f32, kind="ExternalInput").ap()
    bvb_d = nc.dram_tensor("bvb", [128, MPC], f32, kind="ExternalInput").ap()
    wo_d = nc.dram_tensor("wo", [MPC, C], bf16, kind="ExternalInput").ap()
    mk_d = nc.dram_tensor("mk", [max(n_masks, 1), 128, QW], f32,
                          kind="ExternalInput").ap()
    out_d = nc.dram_tensor("out", [C, T], bf16, kind="ExternalOutput").ap()
    if DEBUG:
        dbg = {
            "dbg_qt0": nc.dram_tensor("dbg_qt0", [128, QW], bf16,
                                      kind="ExternalOutput").ap(),
            "dbg_kt0": nc.dram_tensor("dbg_kt0", [128, T], bf16,
                                      kind="ExternalOutput").ap(),
            "dbg_vsb0": nc.dram_tensor("dbg_vsb0", [128, HPC * (DH + 1)],
                                       bf16, kind="ExternalOutput").ap(),
            "dbg_vs2": nc.dram_tensor("dbg_vs2", [128, HPC * 2 * 128], fp8,
                                      kind="ExternalOutput").ap(),
            "dbg_at0": nc.dram_tensor("dbg_at0", [128, NM_T * QW], bf16,
                                      kind="ExternalOutput").ap(),
            "dbg_p0": nc.dram_tensor("dbg_p0", [128, 2 * QW], bf16,
                                     kind="ExternalOutput").ap(),
            "dbg_aug0": nc.dram_tensor("dbg_aug0", [DH + 1, QW], f32,
                                       kind="ExternalOutput").ap(),
            "dbg_bc": nc.dram_tensor("dbg_bc", [DH, QW], f32,
                                     kind="ExternalOutput").ap(),
            "dbg_rec": nc.dram_tensor("dbg_rec", [1, QW], f32,
                                      kind="ExternalOutput").ap(),
            "dbg_augs": nc.dram_tensor("dbg_augs", [DH + 1, QW], f32,
                                       kind="ExternalOutput").ap(),
        }

    esc = float(1.0 / np.sqrt(DH))

    with tile.TileContext(nc) as tctx:
        with ExitStack() as ctx:
            cons = ctx.enter_context(tctx.tile_pool(name="cons", bufs=1))
            store = ctx.enter_context(tctx.tile_pool(name="store", bufs=1))
            wp = ctx.enter_context(tctx.tile_pool(name="wqkv", bufs=1))
            xp = ctx.enter_context(tctx.tile_pool(name="xin", bufs=2))
            qtp = ctx.enter_context(tctx.tile_pool(name="qtc", bufs=2))
            atp = ctx.enter_context(tctx.tile_pool(name="atc", bufs=2))
            pp1 = ctx.enter_context(tctx.tile_pool(name="ps1", bufs=2,
                                                   space="PSUM"))
            spp = ctx.enter_context(tctx.tile_pool(name="ps2", bufs=2,
                                                   space="PSUM"))
            apl = ctx.enter_context(tctx.tile_pool(name="paug", bufs=1,
                                                   space="PSUM"))
            pxp = ctx.enter_context(tctx.tile_pool(name="pexp", bufs=3))
            npl = ctx.enter_context(tctx.tile_pool(name="norm", bufs=2))
            obp = ctx.enter_context(tctx.tile_pool(name="ob", bufs=3))

            bqk_sb = cons.tile([128, 8], f32, tag="bqk")
            nc.gpsimd.dma_start(bqk_sb[:], bqk_d[:])
            bvb_sb = cons.tile([128, MPC], f32, tag="bvb")
            nc.gpsimd.dma_start(bvb_sb[:], bvb_d[:])
            ones_fd = cons.tile([128, DH], f32, tag="onesfd")
            nc.vector.memset(ones_fd[:], 1.0)
            ones_r = cons.tile([1, DH], mybir.dt.float32r, tag="onesr")
            nc.vector.tensor_copy(ones_r[:], ones_fd[0:1, :])
            mask_f, mask_8, mask_b = [], [], []
            for i in range(n_masks):
                mf = cons.tile([128, QW], f32, tag=f"mf{i}", name=f"mf{i}")
                nc.gpsimd.dma_start(mf[:], mk_d[i])
                m8 = cons.tile([128, QW], fp8, tag=f"m8{i}", name=f"m8{i}")
                nc.vector.tensor_copy(m8[:], mf[:])
                mb = cons.tile([128, QW], bf16, tag=f"mb{i}", name=f"mb{i}")
                nc.vector.tensor_copy(mb[:], mf[:])
                mask_8.append(m8)
                mask_b.append(mb)

            # persistent K^T (bf16) and V stores: fp8 unit-pair layout for
            # chunks>=1, clean bf16 layout for chunk 0's k-tiles
            KT = [store.tile([128, T], bf16, tag=f"kt{i}", name=f"kt{i}")
                  for i in range(NM_T)]
            # [kpos, head, unit-in-pair, padded dh+1]: the AV lhsT slice
            # [:, h, u, 0:65] / [:, h, :, 0:65] needs a 128B-aligned unit
            # stride (walrus crashes on odd weight subtile strides)
            VS2 = [store.tile([128, HPC, 2, 128], fp8, tag=f"vs{i}",
                              name=f"vs{i}") for i in range(NKT // 2)]
            for j2 in range(NKT // 2):
                nc.vector.memset(VS2[j2][:, :, :, DH:DH + 1], 1.0)
            VSB = [store.tile([128, HPC * (DH + 1)], bf16, tag=f"vb{i}",
                              name=f"vb{i}") for i in range(4)]
            ones_f = cons.tile([128, HPC], f32, tag="onesf")
            nc.vector.memset(ones_f[:], 1.0)

            # resident weights
            wq_sb = wp.tile([128, NC_T, MPC], fp8, tag="wq")
            wk_sb = wp.tile([128, NC_T, MPC], fp8, tag="wk")
            wv_sb = wp.tile([128, NC_T, MPC], fp8, tag="wv")
            wvb_sb = wp.tile([128, NC_T, MPC], bf16, tag="wvb")
            wo_sb = wp.tile([128, NM_T, C], bf16, tag="wo")
            x0b_sb = wp.tile([128, NC_T, QW], bf16, tag="x0b")

            def emit_weight_dmas_after_x0():
                for ci in range(NC_T):
                    nc.sync.dma_start(wk_sb[:, ci, :],
                                      wk_d[ci * 128:(ci + 1) * 128, :])
                for ci in range(NC_T):
                    nc.gpsimd.dma_start(wv_sb[:, ci, :],
                                        wv_d[ci * 128:(ci + 1) * 128, :])
                for ci in range(NC_T):
                    nc.gpsimd.dma_start(wvb_sb[:, ci, :],
                                        wvb_d[ci * 128:(ci + 1) * 128, :])
                    nc.gpsimd.dma_start(x0b_sb[:, ci, :],
                                        x0b_d[ci * 128:(ci + 1) * 128, :])
                for ci in range(NM_T):
                    nc.gpsimd.dma_start(wo_sb[:, ci, :],
                                        wo_d[ci * 128:(ci + 1) * 128, :])

            def emit_x_load(tci, with_wq=False):
                t0 = tci * QW
                x_sb = xp.tile([128, NC_T, QW], fp8, tag="x", name=f"x_{tci}")
                for ci in range(NC_T):
                    nc.sync.dma_start(x_sb[:, ci, :],
                                      xt_d[ci * 128:(ci + 1) * 128,
                                           t0:t0 + QW])
                    if with_wq:
                        nc.sync.dma_start(wq_sb[:, ci, :],
                                          wq_d[ci * 128:(ci + 1) * 128, :])
                QTc = [qtp.tile([128, QW], bf16, tag=f"qt{i}",
                                name=f"qt{i}_{tci}") for i in range(NM_T)]
                return x_sb, QTc

            # fp8 hybrid chain: ci0,ci1 non-DR (single start covers cols),
            # then DR pairs. Epilogue divides by WSCALE.
            def emit_qk_chain(tci, mt, x_sb, QTc):
                t0 = tci * QW
                w_sb = wq_sb if mt < NM_T else wk_sb
                col = (mt % NM_T) * 128
                ps = pp1.tile([128, QW], f32, tag="qk", name=f"qk{mt}_{tci}")
                for ci in range(2):
                    nc.tensor.matmul(ps[:], w_sb[:, ci, col:col + 128],
                                     x_sb[:, ci, :],
                                     start=(ci == 0), stop=False)
                for j in range(1, NC_T // 2):
                    wsl = w_sb[:, 2 * j:2 * j + 2, col:col + 128]
                    nc.tensor.ldweights(wsl, perf_mode=DRM)
                    for nh in range(2):
                        nc.tensor.matmul(
                            ps[:, nh * 256:(nh + 1) * 256], wsl,
                            x_sb[:, 2 * j:2 * j + 2,
                                 nh * 256:(nh + 1) * 256],
                            start=False,
                            stop=(j == NC_T // 2 - 1 and nh == 1),
                            perf_mode=DRM)
                dstt = (QTc[mt][:] if mt < NM_T
                        else KT[mt - NM_T][:, t0:t0 + QW])
                nc.vector.tensor_scalar(dstt, ps[:], 1.0 / WSCALE,
                                        bqk_sb[:, mt:mt + 1],
                                        AluOp.mult, AluOp.add)

            def emit_v_chain(tci, g, x_sb):
                kt = tci * 4 + g
                ps = pp1.tile([128, QW], f32, tag="qk", name=f"v{kt}")
                for ci in range(2):
                    nc.tensor.matmul(ps[:], x_sb[:, ci, g * 128:(g + 1) * 128],
                                     wv_sb[:, ci, :],
                                     start=(ci == 0), stop=False)
                for j in range(1, NC_T // 2):
                    xsl = x_sb[:, 2 * j:2 * j + 2, g * 128:(g + 1) * 128]
                    nc.tensor.ldweights(xsl, perf_mode=DRM)
                    for nh in range(2):
                        nc.tensor.matmul(
                            ps[:, nh * 256:(nh + 1) * 256], xsl,
                            wv_sb[:, 2 * j:2 * j + 2,
                                  nh * 256:(nh + 1) * 256],
                            start=False,
                            stop=(j == NC_T // 2 - 1 and nh == 1),
                            perf_mode=DRM)
                nc.vector.scalar_tensor_tensor(
                    VS2[kt // 2][:, :, kt % 2, 0:DH],
                    ps[:].rearrange("p (b c) -> p b c", b=HPC),
                    1.0 / WSCALE,
                    bvb_sb[:].rearrange("p (b c) -> p b c", b=HPC),
                    AluOp.mult, AluOp.add)

            # clean bf16 V chains for chunk 0's k-tiles (kt 0..3)
            def emit_v_chain_bf(g):
                ps = pp1.tile([128, QW], f32, tag="qk", name=f"vb{g}")
                for ci in range(NC_T):
                    nc.tensor.matmul(ps[:],
                                     x0b_sb[:, ci, g * 128:(g + 1) * 128],
                                     wvb_sb[:, ci, :],
                                     start=(ci == 0), stop=(ci == NC_T - 1))
                vv = VSB[g][:].rearrange("p (b c) -> p b c", b=HPC)
                nc.vector.tensor_add(
                    vv[:, :, 0:DH],
                    ps[:].rearrange("p (b c) -> p b c", b=HPC),
                    bvb_sb[:].rearrange("p (b c) -> p b c", b=HPC))
                nc.vector.tensor_copy(
                    vv[:, :, DH:DH + 1],
                    ones_f[:].rearrange("p (a b) -> p a b", b=1))

            def emit_fc_chain(tci, co, ATc):
                t0 = tci * QW
                ps = pp1.tile([128, QW], f32, tag="qk", name=f"o{co}_{tci}")
                for ci in range(NM_T):
                    nc.tensor.matmul(ps[:],
                                     wo_sb[:, ci, co * 128:(co + 1) * 128],
                                     ATc[:, ci, :],
                                     start=(ci == 0), stop=(ci == NM_T - 1))
                ob = obp.tile([128, QW], bf16, tag="ob", name=f"ob{co}_{tci}")
                nc.vector.tensor_copy(ob[:], ps[:])
                nc.sync.dma_start(out_d[co * 128:(co + 1) * 128, t0:t0 + QW],
                                  ob[:])

            # chunk 0 prologue
            x_cur, QT_cur = emit_x_load(0, with_wq=True)
            emit_weight_dmas_after_x0()
            for mt in range(2 * NM_T):
                emit_qk_chain(0, mt, x_cur, QT_cur)
            for g in range(4):
                emit_v_chain(0, g, x_cur)
            for g in range(4):
                emit_v_chain_bf(g)

            if DEBUG:
                nc.sync.dma_start(dbg["dbg_qt0"][:], QT_cur[0][:])

            AT_prev = None
            for tci in range(NQC):
                QTc = QT_cur
                if tci + 1 < NQC:
                    x_nxt, QT_nxt = emit_x_load(tci + 1)
                else:
                    x_nxt, QT_nxt = None, None

                fillers = []
                if x_nxt is not None:
                    for mt in range(2 * NM_T):
                        fillers.append(
                            lambda mt=mt: emit_qk_chain(tci + 1, mt, x_nxt,
                                                        QT_nxt))
                    for g in range(4):
                        fillers.append(
                            lambda g=g: emit_v_chain(tci + 1, g, x_nxt))
                if AT_prev is not None:
                    for co in range(NC_T):
                        fillers.append(
                            lambda co=co: emit_fc_chain(tci - 1, co, AT_prev))
                fillers.reverse()

                # ------------- attention for q-chunk == tci -------------
                fp8_path = tci > 0
                units = units_by_qc[tci]
                groups = _pair_groups(units) if fp8_path else None
                n_steps = len(groups) if fp8_path else len(units)
                ATc = atp.tile([128, NM_T, QW], bf16, tag="at",
                               name=f"at_{tci}")
                qmin = min(u[1] for u in units) if units else 0
                total_slots = max(NM_T * max(n_steps, 1), 1)
                stride = max(1, total_slots // max(1, len(fillers)))
                slot = [0]

                def tick():
                    slot[0] += 1
                    if fillers and slot[0] % stride == 0:
                        fillers.pop()()

                if not units:
                    nc.vector.memset(ATc[:], 0.0)
                    while fillers:
                        fillers.pop()()
                    x_cur, QT_cur = x_nxt, QT_nxt
                    AT_prev = ATc
                    continue

                pending_pe_norm = []
                for pr in range(NM_T):        # head pair = (2pr, 2pr+1)
                    hA, hB = 2 * pr, 2 * pr + 1
                    mt = pr
                    # allocated lazily at first AV so the previous pair's
                    # deferred pe_norm (same banks, bufs=1) is emitted first
                    aug = []

                    def get_aug(pr=pr, tci=tci, aug=aug):
                        if not aug:
                            aug.extend(
                                apl.tile([DH + 1, QW], f32, tag=f"aug{h}",
                                         name=f"aug{h}_{pr}_{tci}")
                                for h in range(2))
                        return aug

                    def emit_scores(unit, dst_ap, mt=mt):
                        kt, qoff, midx, mc0, mc1 = unit
                        sc = spp.tile([128, 2 * QW], f32, tag="s",
                                      name=f"s{mt}_{kt}_{tci}")
                        for half, po in ((0, 0), (1, 64)):
                            c0 = half * QW
                            nc.tensor.matmul(
                                sc[:, c0 + qoff:c0 + QW],
                                KT[mt][po:po + DH, kt * KW:(kt + 1) * KW],
                                QTc[mt][po:po + DH, qoff:QW],
                                start=True, stop=True)
                        nc.scalar.activation(dst_ap[:, qoff:2 * QW],
                                             sc[:, qoff:2 * QW], Exp,
                                             scale=esc)
                        return midx, mc0, mc1

                    # ---------------- bf16 path (chunk 0) ----------------
                    if not fp8_path:
                        n_u = len(units)
                        pt = [None] * n_u

                        def emit_unit_bf(i, pt=pt):
                            unit = units[i]
                            p = pxp.tile([128, 2 * QW], bf16, tag="pb",
                                         name=f"pb{pr}_{i}_{tci}")
                            midx, mc0, mc1 = emit_scores(unit, p[:])
                            if midx is not None:
                                mw = mc1 - mc0
                                pm = p[:].rearrange(
                                    "p (a c) -> p a c", c=QW)[:, :, mc0:mc1]
                                nc.vector.tensor_mul(
                                    pm, pm,
                                    mask_b[midx][:, mc0:mc1].unsqueeze(1)
                                    .broadcast_to([128, 2, mw]))
                            pt[i] = p

                        def emit_av_bf(i, hA=hA, hB=hB, pt=pt,
                                       n_u=n_u, get_aug=get_aug):
                            aug = get_aug()
                            kt, qoff, _, _, _ = units[i]
                            p = pt[i]
                            for half, hh in ((0, hA), (1, hB)):
                                c0 = half * QW
                                nc.tensor.matmul(
                                    aug[half][0:DH + 1, qoff:QW],
                                    VSB[kt][:, hh * (DH + 1):
                                            (hh + 1) * (DH + 1)],
                                    p[:, c0 + qoff:c0 + QW],
                                    start=(i == 0), stop=(i == n_u - 1))

                        for i in range(n_u):
                            emit_unit_bf(i)
                            if DEBUG and tci == 0 and pr == 0 and i == 0:
                                nc.sync.dma_start(dbg["dbg_p0"][:], pt[0][:])
                            if i >= 1:
                                emit_av_bf(i - 1)
                            tick()
                            if i == 0 and pending_pe_norm:
                                pending_pe_norm.pop()()
                        emit_av_bf(n_u - 1)
                        if DEBUG and tci == 0 and pr == 0:
                            ad = npl.tile([DH + 1, QW], f32, tag="dbgaug",
                                          name="dbgaug")
                            nc.vector.tensor_copy(ad[:], aug[0][:])
                            nc.sync.dma_start(dbg["dbg_aug0"][:], ad[:])

                    # ---------------- fp8 path (chunks 1-3) ----------------
                    else:
                        n_g = len(groups)
                        pt = [None] * n_g

                        def emit_group(gi, pr=pr, pt=pt):
                            ua, ub = groups[gi]
                            p_tile = pxp.tile([128, 2, 2, QW], fp8, tag="p8",
                                              name=f"p{pr}_{gi}_{tci}")
                            gq = ua[1]
                            for u in (ua, ub):
                                if u is None:
                                    continue
                                uslot = u[0] % 2
                                pdst = p_tile[:, uslot].rearrange(
                                    "p a b -> p (a b)")
                                midx, mc0, mc1 = emit_scores(u, pdst)
                                if midx is not None:
                                    mw = mc1 - mc0
                                    pm = p_tile[:, uslot, :, mc0:mc1]
                                    nc.vector.tensor_mul(
                                        pm, pm,
                                        mask_8[midx][:, mc0:mc1].unsqueeze(1)
                                        .broadcast_to([128, 2, mw]))
                            if gi > 0:
                                # later DR groups read both unit slots from
                                # gq up: zero cols not covered by a unit
                                if ub is None:
                                    nc.vector.memset(
                                        p_tile[:, 1 - ua[0] % 2, :, gq:QW],
                                        0.0)
                                elif ub[1] > gq:
                                    nc.vector.memset(
                                        p_tile[:, ub[0] % 2, :, gq:ub[1]],
                                        0.0)
                            pt[gi] = (p_tile, gq)

                        def emit_av8(gi, hA=hA, hB=hB, pt=pt,
                                     n_g=n_g, get_aug=get_aug):
                            aug = get_aug()
                            ua, ub = groups[gi]
                            p_tile, gq = pt[gi]
                            j2 = ua[0] // 2
                            last = (gi == n_g - 1)
                            if gi == 0:
                                # single start=True per aug bank: non-DR,
                                # full [gq:QW) span, one matmul per unit
                                for half, hh in ((0, hA), (1, hB)):
                                    for idx, u in enumerate((ua, ub)):
                                        if u is None:
                                            continue
                                        us = u[0] % 2
                                        nc.tensor.matmul(
                                            aug[half][0:DH + 1, gq:QW],
                                            VS2[j2][:, hh, us, 0:DH + 1],
                                            p_tile[:, us, half, gq:QW],
                                            start=(idx == 0),
                                            stop=(last and idx ==
                                                  (1 if ub is not None
                                                   else 0)))
                                return
                            splits = ([gq, 256, QW] if gq < 256
                                      else [gq, QW])
                            for half, hh in ((0, hA), (1, hB)):
                                vsl = VS2[j2][:, hh, :, 0:DH + 1]
                                nc.tensor.ldweights(vsl, perf_mode=DRM)
                                for si, (n0, n1) in enumerate(
                                        zip(splits[:-1], splits[1:])):
                                    nc.tensor.matmul(
                                        aug[half][0:DH + 1, n0:n1], vsl,
                                        p_tile[:, :, half, n0:n1],
                                        start=False,
                                        stop=(last and
                                              si == len(splits) - 2),
                                        perf_mode=DRM)

                        for gi in range(n_g):
                            emit_group(gi)
                            if gi >= 1:
                                emit_av8(gi - 1)
                            tick()
                            if gi == 0 and pending_pe_norm:
                                pending_pe_norm.pop()()
                        emit_av8(n_g - 1)

                    # normalization, DVE part: copy aug out of PSUM,
                    # reciprocal of the denominator row
                    aug = get_aug()
                    augs_l = []
                    den = npl.tile([1, 2 * QW], f32, tag="den",
                                   name=f"den_{pr}_{tci}")
                    for half in range(2):
                        augs = npl.tile([DH + 1, QW], f32,
                                        tag=f"augs{half}",
                                        name=f"augs{half}_{pr}_{tci}")
                        nc.vector.tensor_copy(
                            augs[0:DH + 1, qmin:QW],
                            aug[half][0:DH + 1, qmin:QW])
                        nc.vector.tensor_copy(
                            den[0:1, half * QW + qmin:half * QW + QW],
                            aug[half][DH:DH + 1, qmin:QW])
                        augs_l.append(augs)
                    rec = npl.tile([1, 2 * QW], f32, tag="rec",
                                   name=f"rec_{pr}_{tci}")
                    nc.vector.reciprocal_approx_fast(rec[:], den[:])
                    rec_r = npl.tile([1, 2 * QW], mybir.dt.float32r,
                                     tag="recr", name=f"recr_{pr}_{tci}")
                    nc.vector.tensor_copy(rec_r[:], rec[:])
                    rec_l = [rec_r[0:1, 0:QW], rec_r[0:1, QW:2 * QW]]

                    # normalization, PE part deferred into the next pair:
                    # broadcast 1/den across partitions via a K=1 matmul
                    # into the dead aug PSUM rows, then multiply on DVE
                    def pe_norm(pr=pr, mt=mt, aug=aug, augs_l=augs_l,
                                rec_l=rec_l, hA=hA, hB=hB, qmin=qmin,
                                ATc=ATc, tci=tci):
                        for half, hh in ((0, hA), (1, hB)):
                            nc.tensor.matmul(
                                aug[half][0:DH, qmin:QW],
                                ones_r[0:1, 0:DH],
                                rec_l[half][:, qmin:QW],
                                start=True, stop=True)
                            po = (hh % 2) * DH
                            nc.vector.tensor_mul(
                                ATc[po:po + DH, mt, qmin:QW],
                                augs_l[half][0:DH, qmin:QW],
                                aug[half][0:DH, qmin:QW])
                        if DEBUG and tci == 0 and pr == 0:
                            bc = npl.tile([DH, QW], f32, tag="dbc",
                                          name="dbc")
                            nc.vector.tensor_copy(bc[:], aug[0][0:DH, :])
                            nc.sync.dma_start(dbg["dbg_bc"][:], bc[:])
                            rc = npl.tile([1, QW], f32, tag="drc",
                                          name="drc")
                            nc.vector.tensor_copy(rc[:], rec_l[0][0:1, :])
                            nc.sync.dma_start(dbg["dbg_rec"][:], rc[:])
                            nc.sync.dma_start(dbg["dbg_augs"][:],
                                              augs_l[0][:])
                    pending_pe_norm.append(pe_norm)
                if pending_pe_norm:
                    if fillers:
                        fillers.pop()()
                    pending_pe_norm.pop()()
                while fillers:
                    fillers.pop()()

                if DEBUG and tci == 0:
                    nc.sync.dma_start(
                        dbg["dbg_at0"][:],
                        ATc[:].rearrange("p a b -> p (a b)"))
                x_cur, QT_cur = x_nxt, QT_nxt
                AT_prev = ATc

            # last chunk's fc_out
            for co in range(NC_T):
                emit_fc_chain(NQC - 1, co, AT_prev)
            if DEBUG:
                nc.sync.dma_start(dbg["dbg_kt0"][:], KT[0][:])
                nc.sync.dma_start(dbg["dbg_vsb0"][:], VSB[0][:])
                nc.sync.dma_start(
                    dbg["dbg_vs2"][:],
                    VS2[2][:].rearrange("p a b c -> p (a b c)"))
    nc.compile()
    return nc


def kernel(x, W_qkv, b_qkv, W_out, b_out, mask, _trace=False):
    x = np.asarray(x, dtype=np.float32)
    W_qkv = np.asarray(W_qkv, dtype=np.float32)
    b_qkv = np.asarray(b_qkv, dtype=np.float32)
    W_out = np.asarray(W_out, dtype=np.float32)
    b_out = np.asarray(b_out, dtype=np.float32)
    mask2d = np.asarray(mask).reshape(T, T)

    key = hashlib.sha256(mask2d.tobytes()).hexdigest() + str(DEBUG)
    if key in _prog_cache:
        nc, units_by_qc, mask_tiles = _prog_cache[key]
    else:
        units_by_qc, mask_tiles = _mask_plan(mask2d)
        nc = _build_program(units_by_qc, len(mask_tiles))
        _prog_cache[key] = (nc, units_by_qc, mask_tiles)

    mk = (np.stack(mask_tiles) if mask_tiles
          else np.zeros((1, 128, QW), np.float32))

    in_maps = []
    for c in range(NCORES):
        b, hg = c // 2, c % 2
        r = slice(hg * MPC, (hg + 1) * MPC)
        xtf = np.ascontiguousarray(x[b].T)                      # [1024, 2048]
        xt = xtf.astype(E4)
        x0b = xtf[:, 0:QW].astype(ml_dtypes.bfloat16)
        wqf = np.ascontiguousarray(W_qkv[0 * C:1 * C][r].T)     # [1024, 512]
        wkf = np.ascontiguousarray(W_qkv[1 * C:2 * C][r].T)
        wvf = np.ascontiguousarray(W_qkv[2 * C:3 * C][r].T)
        wq = (wqf * WSCALE).astype(E4)
        wk = (wkf * WSCALE).astype(E4)
        wv = (wvf * WSCALE).astype(E4)
        wvb = wvf.astype(ml_dtypes.bfloat16)
        bq = b_qkv[0 * C:1 * C][r]
        bk = b_qkv[1 * C:2 * C][r]
        bv = b_qkv[2 * C:3 * C][r]
        bqk = np.concatenate([bq.reshape(4, 128).T, bk.reshape(4, 128).T],
                             axis=1)                            # [128, 8]
        bvb = np.tile(bv, (128, 1))                             # [128, 512]
        wo = np.ascontiguousarray(W_out[:, r].T).astype(ml_dtypes.bfloat16)
        in_maps.append({
            "xt": xt, "x0b": x0b, "wq": wq, "wk": wk, "wv": wv, "wvb": wvb,
            "bqk": np.ascontiguousarray(bqk), "bvb": bvb,
            "wo": wo, "mk": mk,
        })

    res = run_bass_kernel_spmd(nc, in_maps, core_ids=list(range(NCORES)),
                               trace=_trace)
    out = np.empty((B, T, C), np.float32)
    for b in range(B):
        out[b] = (np.asarray(res.results[2 * b]["out"], dtype=np.float32)
                  + np.asarray(res.results[2 * b + 1]["out"],
                               dtype=np.float32)).T + b_out
    if _trace:
        kernel.last_result = res
    return out
